# revision 3
# baseline (speedup 1.0000x reference)
"""Trainium2 Bass kernel for nn_AttentionBlock — v3.

v3 over v2: 3-deep score-psum ring (PSUM = 12KB ring + 4KB pav, exactly 16KB;
k/v/q/proj/GN psum borrows ring slots), merged single GroupNorm reduce chain,
PE warm-up choreographed around the GN matmuls, out-DMAs off Pool.
"""

import math
from collections import deque
from contextlib import ExitStack

import numpy as np

import concourse.bacc as bacc
import concourse.bass as bass
import concourse.mybir as mybir
import concourse.tile as tile

F32 = mybir.dt.float32
F32R = mybir.dt.float32r
BF16 = mybir.dt.bfloat16
F8E4 = mybir.dt.float8e4
AF = mybir.ActivationFunctionType
ALU = mybir.AluOpType
PM = mybir.MatmulPerfMode

C = 256
T = 4096
NH = 8
CHD = 32
NCORES = 8
TC = T // NCORES
NSB = T // 128
NPAIR = NSB // 2
EPS = 1e-5
SCALE2 = 1.0 / math.sqrt(CHD)
NSUB = T // 512
CSHIFT = 3.0
LAG = 8
VSL = 48

# exp slots on Pool (rest on ACT), 5 per head: heads 0/1 are interleaved
# slot-wise, so their pool pairs are disjoint to avoid back-to-back Pool
# slots; head 7 keeps its tail pairs on ACT.
POOL_PAIRS_H0 = {1, 4, 7, 10, 13}
POOL_PAIRS_H1 = {2, 5, 8, 11, 14}
POOL_PAIRS_MID = {1, 4, 7, 10, 13}
POOL_PAIRS_LAST = {1, 4, 7, 10}


def build_nc():
    nc = bacc.Bacc(trn_type="TRN2")

    x_bf = nc.dram_tensor("x_bf", [C, T], BF16, kind="ExternalInput")
    x_c = nc.dram_tensor("x_c", [C, TC], F32, kind="ExternalInput")
    w_q = nc.dram_tensor("w_q", [C, 384], F32R, kind="ExternalInput")
    w_kv = nc.dram_tensor("w_kv", [C, 640], BF16, kind="ExternalInput")
    w_p4 = nc.dram_tensor("w_p4", [128, 512], F32R, kind="ExternalInput")
    bcat = nc.dram_tensor("bcat", [128, 9], F32, kind="ExternalInput")
    gmask = nc.dram_tensor("gmask", [128, 4], F32, kind="ExternalInput")
    gmaskT = nc.dram_tensor("gmaskT", [4, 128], F32, kind="ExternalInput")
    out = nc.dram_tensor("out", [C, TC], F32, kind="ExternalOutput")

    with tile.TileContext(nc) as tc, ExitStack() as ctx:
        xbp = ctx.enter_context(tc.tile_pool(name="xbp", bufs=1))
        xnp = ctx.enter_context(tc.tile_pool(name="xnp", bufs=1))
        kp = ctx.enter_context(tc.tile_pool(name="kp", bufs=1))
        cst = ctx.enter_context(tc.tile_pool(name="cst", bufs=1))
        med = ctx.enter_context(tc.tile_pool(name="med", bufs=1))
        sm = ctx.enter_context(tc.tile_pool(name="sm", bufs=2))
        pex = ctx.enter_context(tc.tile_pool(name="pex", bufs=12))
        stp = ctx.enter_context(tc.tile_pool(name="stp", bufs=7))
        rbp = ctx.enter_context(tc.tile_pool(name="rbp", bufs=3))
        dscr = ctx.enter_context(tc.tile_pool(name="dscr", bufs=2, space="DRAM"))
        ps_s = ctx.enter_context(tc.tile_pool(name="ps_s", bufs=3, space="PSUM"))
        ps_a = ctx.enter_context(tc.tile_pool(name="ps_a", bufs=2, space="PSUM"))

        def ring():
            # every psum need goes through the 3-deep [128, 1024] ring
            return ps_s.tile([128, 2 * TC], F32, tag="ps_s", name="ring")

        # ---- x loads ----
        xt = [xbp.tile([128, T], BF16, tag=f"xt{j}", name=f"xt{j}") for j in range(2)]
        xct = [sm.tile([128, TC], F32, tag=f"xct{j}", bufs=1, name=f"xct{j}") for j in range(2)]
        for cch in range(4):
            cs = slice(T // 4 * cch, T // 4 * (cch + 1))
            nc.sync.dma_start(out=xt[0][:, cs], in_=x_bf[0:128, cs])
            nc.scalar.dma_start(out=xt[1][:, cs], in_=x_bf[128:256, cs])
        for j in range(2):
            nc.scalar.dma_start(out=xct[j], in_=x_c[128 * j:128 * (j + 1), :])

        # ---- constants ----
        wq_sb = [cst.tile([128, 384], F32R, tag=f"wq{j}", name=f"wq{j}") for j in range(2)]
        wkv_sb = [cst.tile([128, 640], BF16, tag=f"wkv{j}", name=f"wkv{j}") for j in range(2)]
        wp4_sb = cst.tile([128, 512], F32R, tag="wp4", name="wp4")
        bc_sb = cst.tile([128, 9], F32, tag="bc", name="bc")
        mk_sb = cst.tile([128, 4], F32, tag="mk", name="mk")
        mkT_sb = cst.tile([4, 128], F32, tag="mkT", name="mkT")
        nc.sync.dma_start(out=mk_sb, in_=gmask[:])
        nc.sync.dma_start(out=mkT_sb, in_=gmaskT[:])
        nc.sync.dma_start(out=bc_sb, in_=bcat[:])
        for j in range(2):
            r = slice(128 * j, 128 * (j + 1))
            nc.sync.dma_start(out=wkv_sb[j], in_=w_kv[r, :])
            nc.sync.dma_start(out=wq_sb[j], in_=w_q[r, :])
        nc.sync.dma_start(out=wp4_sb, in_=w_p4[:])

        econst = cst.tile([128, 2 * TC], F32, tag="econst", name="econst")
        negc = cst.tile([128, 1], F32, tag="negc", name="negc")
        ebf = cst.tile([4, TC], BF16, tag="ebf", name="ebf")
        nc.gpsimd.memset(econst, float(np.e))
        nc.gpsimd.memset(negc, -CSHIFT)
        with nc.allow_low_precision(reason="bf16 warmup const"):
            nc.vector.memset(ebf, 1.0)

        vt8 = med.tile([128, NSB, NH * VSL], F8E4, tag="vt", name="vt")
        ones8 = cst.tile([128, NSB * NH], F8E4, tag="ones8", name="ones8")
        with nc.allow_low_precision(reason="fp8 ones"):
            nc.vector.memset(ones8, 1.0)
            nc.vector.tensor_copy(
                out=vt8[:].rearrange("p s (h c) -> p (s h) c", c=VSL)[:, :, 32:33],
                in_=ones8[:].rearrange("p (g c) -> p g c", c=1))

        def dummy_f32(width=TC):
            pd = ring()
            nc.tensor.matmul(pd[0:4, 0:width], econst[:, 0:4],
                             econst[:, 0:width], start=True, stop=True)

        def dummy_bf16(width=TC):
            pd = ring()
            nc.tensor.matmul(pd[0:4, 0:width], ebf[0:4, 0:4],
                             ebf[0:4, 0:width], start=True, stop=True)

        # ---- PE warm-up (p-state ramp) ----
        for _ in range(5):
            dummy_f32()

        # ---- GroupNorm stats ----
        # DVE bn_stats: tile 0 fully, tile 1 subs 0-4; ACT accumulates
        # sum(x)/sum(x^2) over tile-1 cols 2560:4096 in parallel.
        stat = sm.tile([128, 2, 2], F32, tag="stat", bufs=1, name="stat")
        sxa = sm.tile([128, 2], F32, tag="sxa", bufs=1, name="sxa")
        scr = sm.tile([128, 1536], F32, tag="scr", bufs=1, name="scr")
        nc.scalar.activation(out=scr[:], in_=xt[1][:, 2560:4096],
                             func=AF.Identity, accum_out=sxa[:, 0:1])
        nc.scalar.activation(out=scr[:], in_=xt[1][:, 2560:4096],
                             func=AF.Square, accum_out=sxa[:, 1:2])
        for j in range(2):
            nsb_j = NSUB if j == 0 else 5
            bstat = sm.tile([128, NSUB, 6], F32, tag="bstat", name="bstat")
            xsub = xt[j][:].rearrange("p (s f) -> p s f", f=512)
            for s in range(nsb_j):
                nc.vector.bn_stats(out=bstat[:, s, :], in_=xsub[:, s, :])
            mv = sm.tile([128, 2], F32, tag="mv", name="mv")
            nc.vector.bn_aggr(out=mv[:], in_=bstat[:, 0:nsb_j, :])
            if j == 0:
                nc.vector.tensor_copy(out=stat[:, j, 0:1], in_=mv[:, 0:1])
                nc.vector.tensor_mul(out=stat[:, j, 1:2], in0=mv[:, 0:1], in1=mv[:, 0:1])
                nc.vector.tensor_add(out=stat[:, j, 1:2], in0=stat[:, j, 1:2], in1=mv[:, 1:2])
            else:
                # combine: stat = (2560*bn + act_sum) / 4096
                e2bn = sm.tile([128, 1], F32, tag="e2bn", name="e2bn")
                nc.vector.tensor_mul(out=e2bn[:], in0=mv[:, 0:1], in1=mv[:, 0:1])
                nc.vector.tensor_add(out=e2bn[:], in0=e2bn[:], in1=mv[:, 1:2])
                sxs = sm.tile([128, 2], F32, tag="sxs", name="sxs")
                nc.vector.tensor_scalar_mul(out=sxs[:], in0=sxa[:], scalar1=1.0 / 4096.0)
                nc.vector.scalar_tensor_tensor(
                    out=stat[:, j, 0:1], in0=mv[:, 0:1], scalar=2560.0 / 4096.0,
                    in1=sxs[:, 0:1], op0=ALU.mult, op1=ALU.add)
                nc.vector.scalar_tensor_tensor(
                    out=stat[:, j, 1:2], in0=e2bn[:], scalar=2560.0 / 4096.0,
                    in1=sxs[:, 1:2], op0=ALU.mult, op1=ALU.add)

        pst8 = ring()     # [4, 4]: cols (j, stat)
        for j in range(2):
            nc.tensor.matmul(pst8[0:4, 2 * j:2 * j + 2], mk_sb[:], stat[:, j, :],
                             start=True, stop=True)
        dummy_f32()       # keep PE busy through the Newton chain
        mm8 = sm.tile([4, 2, 2], F32, tag="mm8", name="mm8")
        nc.vector.tensor_scalar_mul(
            out=mm8[:].rearrange("p j s -> p (j s)"), in0=pst8[0:4, 0:4],
            scalar1=1.0 / 32.0)
        var8 = sm.tile([4, 2], F32, tag="var8", name="var8")
        nc.vector.tensor_mul(out=var8[:], in0=mm8[:, :, 0], in1=mm8[:, :, 0])
        nc.vector.tensor_sub(out=var8[:], in0=mm8[:, :, 1], in1=var8[:])
        nc.vector.tensor_scalar_add(out=var8[:], in0=var8[:], scalar1=EPS)
        bcf = sm.tile([4, 2, 2], F32, tag="bcf", name="bcf")   # (istd, mean)
        iv8 = sm.tile([4, 2], F32, tag="iv8", name="iv8")
        nc.vector.reciprocal(out=iv8[:], in_=var8[:])
        nc.scalar.activation(out=bcf[:, :, 0], in_=iv8[:], func=AF.Sqrt)
        nc.vector.tensor_copy(out=bcf[:, :, 1], in_=mm8[:, :, 0])
        A_sb, B_sb = [], []
        chims = []
        for j in range(2):
            chim = ring()
            nc.tensor.matmul(chim[:, 0:2], mkT_sb[:], bcf[:, j, :],
                             start=True, stop=True)
            chims.append(chim)
        dummy_bf16()
        for j in range(2):
            chim = chims[j]
            A = sm.tile([128, 1], F32, tag=f"A{j}", bufs=1, name=f"A{j}")
            B = sm.tile([128, 1], F32, tag=f"B{j}", bufs=1, name=f"B{j}")
            nc.vector.tensor_mul(out=A[:], in0=chim[:, 0:1], in1=bc_sb[:, 3 + j:4 + j])
            tmp = sm.tile([128, 1], F32, tag="tmpB", name="tmpB")
            nc.vector.tensor_mul(out=tmp[:], in0=chim[:, 1:2], in1=A[:])
            nc.vector.tensor_sub(out=B[:], in0=bc_sb[:, 5 + j:6 + j], in1=tmp[:])
            A_sb.append(A)
            B_sb.append(B)

        # ---- xnc (ACT) + q ----
        xnc = [sm.tile([128, TC], F32R, tag=f"xnc{j}", bufs=1, name=f"xnc{j}") for j in range(2)]
        nc.scalar.activation(out=xnc[0][:], in_=xct[0][:], func=AF.Identity,
                             bias=B_sb[0][:], scale=A_sb[0][:])
        with nc.allow_low_precision(reason="f32r xnc"):
            nc.vector.tensor_scalar(out=xnc[1][:], in0=xct[1][:], scalar1=A_sb[1][:],
                                    scalar2=B_sb[1][:], op0=ALU.mult, op1=ALU.add)
        q_sb = [sm.tile([128, TC], F32R, tag=f"q{j}", bufs=1, name=f"q{j}") for j in range(3)]
        for o in range(3):
            pq = ring()
            for kc in range(2):
                nc.tensor.matmul(pq[:, 0:TC], wq_sb[kc][:, 128 * o:128 * (o + 1)],
                                 xnc[kc][:], start=(kc == 0), stop=(kc == 1))
            nc.scalar.activation(out=q_sb[o][:], in_=pq[:, 0:TC], func=AF.Identity,
                                 bias=bc_sb[:, o:o + 1], scale=SCALE2)

        # ---- xn (bf16, 4x DVE) ----
        xn = [xnp.tile([128, T], BF16, tag=f"xn{j}", name=f"xn{j}") for j in range(2)]
        with nc.allow_low_precision(reason="bf16 xn"):
            for cch in range(4):
                cs = slice(1024 * cch, 1024 * (cch + 1))
                for j in range(2):
                    nc.vector.tensor_scalar(
                        out=xn[j][:, cs], in0=xt[j][:, cs], scalar1=A_sb[j][:],
                        scalar2=B_sb[j][:], op0=ALU.mult, op1=ALU.add)

        # ---- k / v production (psum borrowed from the ring) ----
        k_sb = [kp.tile([128, T], F32R, tag=f"k{o}", name=f"k{o}") for o in range(3)]

        def emit_k_chunk(o, nchunk):
            cs = slice(512 * nchunk, 512 * (nchunk + 1))
            pk = ring()
            for kc in range(2):
                nc.tensor.matmul(pk[:, 0:512], wkv_sb[kc][:, 128 * o:128 * (o + 1)],
                                 xn[kc][:, cs], start=(kc == 0), stop=(kc == 1))
            nc.vector.tensor_copy(out=k_sb[o][:, cs], in_=pk[:, 0:512])

        def emit_v_pair(u):
            pv = ring()
            pvv = pv[:, 0:512].rearrange("p (two f) -> p two f", two=2)
            for half in range(2):
                sb = 2 * u + half
                for kc in range(2):
                    nc.tensor.matmul(pvv[:, half, :],
                                     xn[kc][:, 128 * sb:128 * (sb + 1)],
                                     wkv_sb[kc][:, 384:640],
                                     start=(kc == 0), stop=(kc == 1))
            with nc.allow_low_precision(reason="fp8 v"):
                nc.vector.tensor_copy(
                    out=vt8[:, 2 * u:2 * u + 2, :]
                    .rearrange("p two (h c) -> p two h c", c=VSL)[:, :, :, 0:32],
                    in_=pvv.rearrange("p two (h c) -> p two h c", c=32))

        for nchunk in range(2):
            emit_k_chunk(0, nchunk)

        # ---- slot schedule ----
        slot_seq = []
        for p in range(NPAIR):
            slot_seq.append((0, p))
            slot_seq.append((1, p))
        for h in range(2, NH):
            for p in range(NPAIR):
                slot_seq.append((h, p))

        prod_for = {}
        for u in range(NPAIR):                    # v pairs at odd slots
            prod_for[2 * u + 1] = ("v", u)
        for c in range(2, NSUB):                  # k tile 0 chunks 2-7
            prod_for[4 * c - 6] = ("k", (0, c))
        for n in range(16):                       # k tiles 1-2
            prod_for[26 + 2 * n] = ("k", (1 + n // 8, n % 8))

        hout = [sm.tile([128, TC], F32, tag=f"ho{j}", bufs=1, name=f"ho{j}") for j in range(2)]
        at4 = [sm.tile([128, TC], F32R, tag=f"at4{g}", bufs=1, name=f"at4{g}") for g in range(2)]
        hout_inited = [False]

        def init_hout():
            if not hout_inited[0]:
                hout_inited[0] = True
                for o in range(2):
                    nc.vector.tensor_scalar_add(
                        out=hout[o][:], in0=xct[o][:], scalar1=bc_sb[:, 7 + o:8 + o])

        def emit_proj(grp):
            init_hout()
            for o in range(2):
                ppm = ring()
                nc.tensor.matmul(
                    ppm[:, 0:TC],
                    wp4_sb[:, 256 * grp + 128 * o:256 * grp + 128 * (o + 1)],
                    at4[grp][:], start=True, stop=True)
                nc.vector.tensor_add(out=hout[o][:], in0=hout[o][:], in1=ppm[:, 0:TC])
                if grp == 1:
                    eng = nc.sync if o == 0 else nc.scalar
                    eng.dma_start(out=out[128 * o:128 * (o + 1), :], in_=hout[o][:])

        def emit_head_tail(h, pav, last=False):
            grp, hh = h // 4, h % 4
            with nc.allow_low_precision(reason="f32r normalize"):
                rec = sm.tile([1, TC], F32R, tag="rec", name="rec")
                nc.vector.reciprocal(out=rec[:], in_=pav[32:33, :])
                rbt = rbp.tile([32, TC], F32R, tag="rb", name="rb")
                nc.gpsimd.partition_broadcast(rbt[:], rec[:])
                nc.vector.tensor_mul(out=at4[grp][32 * hh:32 * hh + 32, :],
                                     in0=pav[0:32, :], in1=rbt[:])
            if h == 3 or h == 7:
                emit_proj(grp)

        pavs = {}
        pendq = deque()
        tails = []

        def emit_av(pe_t, hp, pp):
            if pp == 0:
                pavs[hp] = ps_a.tile([VSL, TC], F32, tag="ps_a", name="pav")
            nc.tensor.matmul(
                pavs[hp][:],
                vt8[:, 2 * pp:2 * pp + 2, VSL * hp:VSL * (hp + 1)],
                pe_t[:],
                start=(pp == 0), stop=(pp == NPAIR - 1),
                perf_mode=PM.DoubleRow)

        for g, (h, p) in enumerate(slot_seq):
            if g == 40:
                init_hout()
            while tails and g - tails[0][1] >= 2:
                th, _ = tails.pop(0)
                emit_head_tail(th, pavs.pop(th))
            oh, rh = h // 3, 32 * (h % 3)
            pss = ring()
            for half in range(2):
                i = 2 * p + half
                nc.tensor.matmul(
                    pss[:, half * TC:(half + 1) * TC],
                    k_sb[oh][rh:rh + 32, 128 * i:128 * (i + 1)],
                    q_sb[oh][rh:rh + 32, :],
                    start=True, stop=True)
            if len(pendq) >= LAG:
                pe_prev, hp, pp = pendq.popleft()
                emit_av(pe_prev, hp, pp)
                if pp == NPAIR - 1:
                    tails.append((hp, g))
            if h == 0:
                pool_set = POOL_PAIRS_H0
            elif h == 1:
                pool_set = POOL_PAIRS_H1
            elif h <= 6:
                pool_set = POOL_PAIRS_MID
            else:
                pool_set = POOL_PAIRS_LAST
            pe_t = pex.tile([128, 2, TC], F8E4, tag="pex", name="pex")
            with nc.allow_low_precision(reason="fp8 softmax weights"):
                if p in pool_set:
                    stg = stp.tile([128, 2 * TC], F32, tag="stg", name="stg")
                    nc.vector.tensor_scalar_add(out=stg[:], in0=pss[:],
                                                scalar1=negc[:])
                    nc.gpsimd.tensor_tensor(
                        out=pe_t[:].rearrange("p two f -> p (two f)"),
                        in0=econst[:], in1=stg[:], op=ALU.pow)
                else:
                    nc.scalar.activation(
                        out=pe_t[:].rearrange("p two f -> p (two f)"),
                        in_=pss[:], func=AF.Exp, bias=negc[:])
            pendq.append((pe_t, h, p))
            if h == NH - 1 and p >= NPAIR - 5 and len(pendq) > 2:
                pe_prev, hp, pp = pendq.popleft()
                emit_av(pe_prev, hp, pp)
                if pp == NPAIR - 1:
                    tails.append((hp, g))
            unit = prod_for.get(g)
            if unit is not None:
                kind, arg = unit
                if kind == "v":
                    emit_v_pair(arg)
                else:
                    emit_k_chunk(*arg)

        g = len(slot_seq)
        while pendq:
            pe_prev, hp, pp = pendq.popleft()
            emit_av(pe_prev, hp, pp)
            if pp == NPAIR - 1:
                tails.append((hp, g))
            g += 1
        while tails:
            th, _ = tails.pop(0)
            emit_head_tail(th, pavs.pop(th), last=(th == NH - 1))

    nc.compile()
    return nc


def host_prep(inputs):
    import ml_dtypes
    x = np.ascontiguousarray(inputs["x"].reshape(C, T), dtype=np.float32)
    qkv_w = np.asarray(inputs["qkv_w"], dtype=np.float32)
    qkv_b = np.asarray(inputs["qkv_b"], dtype=np.float32)
    proj_w = np.asarray(inputs["proj_w"], dtype=np.float32)
    proj_b = np.asarray(inputs["proj_b"], dtype=np.float32)

    def permute_qk(wT, b):
        wp = np.zeros((C, 384), dtype=np.float32)
        bp = np.zeros((384,), dtype=np.float32)
        for h in range(NH):
            dst = 128 * (h // 3) + 32 * (h % 3)
            wp[:, dst:dst + 32] = wT[:, 32 * h:32 * h + 32]
            bp[dst:dst + 32] = b[32 * h:32 * h + 32]
        return wp, bp

    w_qT, b_qp = permute_qk(qkv_w[0:C].T, qkv_b[0:C] * SCALE2)
    w_kT, _ = permute_qk(qkv_w[C:2 * C].T, qkv_b[C:2 * C])
    w_vT = qkv_w[2 * C:3 * C].T
    w_kv = np.concatenate([w_kT, w_vT], axis=1)

    W = proj_w.reshape(2, 128, 2, 4, CHD)           # [o, j, g, hh, c]
    w_p4 = np.ascontiguousarray(W.transpose(3, 4, 2, 0, 1).reshape(128, 512))

    b_p = proj_b + proj_w @ qkv_b[2 * C:3 * C]
    bcat = np.zeros((128, 9), dtype=np.float32)
    for j in range(3):
        bcat[:, j] = b_qp[128 * j:128 * (j + 1)]
    gn_gamma = np.asarray(inputs["gn_gamma"], np.float32)
    gn_beta = np.asarray(inputs["gn_beta"], np.float32)
    for j in range(2):
        bcat[:, 3 + j] = gn_gamma[128 * j:128 * (j + 1)]
        bcat[:, 5 + j] = gn_beta[128 * j:128 * (j + 1)]
        bcat[:, 7 + j] = b_p[128 * j:128 * (j + 1)]

    gmask = np.zeros((128, 4), dtype=np.float32)
    for p in range(128):
        gmask[p, p // 32] = 1.0
    gmaskT = np.ascontiguousarray(gmask.T)

    shared = {
        "x_bf": x.astype(ml_dtypes.bfloat16),
        "w_q": w_qT, "w_kv": w_kv.astype(ml_dtypes.bfloat16),
        "w_p4": w_p4, "bcat": bcat,
        "gmask": gmask, "gmaskT": gmaskT,
    }
    in_maps = []
    for cid in range(NCORES):
        m = dict(shared)
        m["x_c"] = np.ascontiguousarray(x[:, TC * cid:TC * (cid + 1)])
        in_maps.append(m)
    return in_maps


_NC_CACHE = None


def kernel(**inputs):
    global _NC_CACHE
    from concourse.bass_utils import run_bass_kernel_spmd

    if _NC_CACHE is None:
        _NC_CACHE = build_nc()
    in_maps = host_prep(inputs)
    res = run_bass_kernel_spmd(_NC_CACHE, in_maps, core_ids=list(range(NCORES)))
    outs = [np.asarray(r["out"]) for r in res.results]
    full = np.concatenate(outs, axis=1).reshape(1, C, 64, 64)
    return full.astype(np.float32)
